# revision 1
# baseline (speedup 1.0000x reference)
"""Contrastive loss on 8 Trainium2 NeuronCores (Bass/Tile).

loss * n = sum_ij [ same_ij * (s<1)(1-s) + (1-same_ij) * (s>0.3) * s ],
s = <x_i, x_j>.

Decomposition used here (exact):
    loss * n = sum_ij b(s) + sum_ij same_ij * (relu(1-s) - b(s)),
    b(s) = (s > 0.3) * s.

Strategy:
  * Host: sort rows by label -> same-label pairs live in a narrow diagonal
    band (|i-j| < maxrun). Cast X^T to bf16.
  * Shard rows across 8 cores (1024 rows each). Each core receives a
    column-ROLLED copy of X^T so its own row-slab is always at columns
    0..1023 -> one SPMD program for all cores.
  * Device: S-slab [1024, 8192] via bf16 matmuls (PSUM fp32). Each
    [128,1024] S tile is copied PSUM->SBUF as bf16 (copies split between
    ScalarE and VectorE for engine balance), then one fused DVE op
    (scalar_tensor_tensor) computes b = (S>margin)*S with an accumulated
    per-row sum. Same-label corrections run only on the few band tiles
    straddling the diagonal, using an exact label-equality mask.
  * Host: fp64 sum of per-core accumulator vectors, divide by n.
"""

import numpy as np
import ml_dtypes

import concourse.bass as bass
import concourse.mybir as mybir
from concourse import bacc
import concourse.tile as tile
from concourse.bass_utils import run_bass_kernel_spmd

N_TOTAL = 8192
D = 256
N_CORES = 8
ROWS = N_TOTAL // N_CORES          # 1024 rows per core
M_TILES = ROWS // 128              # 8 partition tiles per core
DT_W = 1024                        # "double tile": 2 PSUM banks wide
N_DT = N_TOTAL // DT_W             # 8 double tiles across columns
MARGIN = 0.3
F32 = mybir.dt.float32
BF16 = mybir.dt.bfloat16

# number of (of 64) S double-tiles handled entirely on ScalarE via
# relu+sign accumulation (no SBUF copy, no DVE work). The rest get an
# ScalarE PSUM->SBUF copy + one fused DVE op. Tuned for engine balance.
RELU_TILES = 8


def _band_windows(pad):
    """Band windows in rolled column space, one entry per (mt, dt) slice:
    (mt, dt, lo, w, tcb_off). tcb region A = cols [0, 1024+pad),
    region B = cols [N-pad, N) stored at offset 1024+pad."""
    a_len = DT_W + pad
    wins = []
    for mt in range(M_TILES):
        c0 = mt * 128 - pad
        c1 = mt * 128 + 128 + pad
        ivs = []
        if c0 < 0:
            ivs.append((N_TOTAL + c0, N_TOTAL))
            c0 = 0
        ivs.append((c0, c1))
        for a, b in ivs:
            for dt in range(a // DT_W, (b - 1) // DT_W + 1):
                lo = max(a, dt * DT_W) - dt * DT_W
                hi = min(b, (dt + 1) * DT_W) - dt * DT_W
                col = dt * DT_W + lo
                if col < a_len:
                    tco = col
                else:
                    assert col >= N_TOTAL - pad
                    tco = a_len + (col - (N_TOTAL - pad))
                wins.append((mt, dt, lo, hi - lo, tco))
    return wins, a_len


def _main_body(nc, tc, psum, spool, bpool, wpool, xk, tcb, trows, accD,
               accE, bias_nm, rset, stt_col, colR, colS, colA, colB,
               wins_by_td, AL, ACT):
    for mt in range(M_TILES):
        lhs = [xk[k][:, mt * 128:(mt + 1) * 128] for k in range(2)]
        for g in range(N_DT // 2):
            dts = (2 * g, 2 * g + 1)
            T = [psum.tile([128, DT_W], F32, name="S") for _ in range(2)]
            for k in range(2):
                for j in range(2):
                    for h in range(2):
                        ntc = dts[j] * DT_W + h * 512
                        nc.tensor.matmul(
                            T[j][:, h * 512:(h + 1) * 512],
                            lhs[k],
                            xk[k][:, ntc:ntc + 512],
                            start=(k == 0),
                            stop=(k == 1),
                        )
            for j in range(2):
                td = (mt, dts[j])
                if td in rset:
                    # all-ScalarE tile: sum relu(S-m) and sum sign(S-m)
                    jr = spool.tile([128, DT_W], BF16, name="jnk")
                    nc.scalar.activation(
                        out=jr[:], in_=T[j][:], func=ACT.Relu,
                        bias=bias_nm[:], scale=1.0,
                        accum_out=accE[:, colR[td]:colR[td] + 1],
                    )
                    js = spool.tile([128, DT_W], BF16, name="jnk")
                    nc.scalar.activation(
                        out=js[:], in_=T[j][:], func=ACT.Sign,
                        bias=bias_nm[:], scale=1.0,
                        accum_out=accE[:, colS[td]:colS[td] + 1],
                    )
                    continue
                # copy S PSUM -> SBUF bf16 on ScalarE
                Sb = spool.tile([128, DT_W], BF16, name="scp")
                nc.scalar.activation(
                    out=Sb[:], in_=T[j][:], func=ACT.Copy,
                    bias=0.0, scale=1.0,
                )
                # b = (S > margin) * S ; accum = row-sum(b)
                bt = bpool.tile([128, DT_W], BF16, name="btile")
                nc.vector.scalar_tensor_tensor(
                    out=bt[:],
                    in0=Sb[:],
                    scalar=MARGIN,
                    in1=Sb[:],
                    op0=AL.is_gt,
                    op1=AL.mult,
                    accum_out=accD[:, stt_col[td]:stt_col[td] + 1],
                )
                for (wi, lo, w, tco) in wins_by_td.get(td, []):
                    m = wpool.tile([128, w], BF16, name="mask")
                    nc.vector.tensor_scalar(
                        out=m[:],
                        in0=tcb[:, tco:tco + w],
                        scalar1=trows[:, mt:mt + 1],
                        scalar2=None,
                        op0=AL.is_equal,
                    )
                    at = wpool.tile([128, w], BF16, name="atile")
                    nc.scalar.activation(
                        out=at[:],
                        in_=Sb[:, lo:lo + w],
                        func=ACT.Relu,
                        bias=1.0,
                        scale=-1.0,
                    )
                    ja = wpool.tile([128, w], BF16, name="junka")
                    nc.vector.scalar_tensor_tensor(
                        out=ja[:],
                        in0=at[:],
                        scalar=0.0,
                        in1=m[:],
                        op0=AL.add,
                        op1=AL.mult,
                        accum_out=accD[:, colA[wi]:colA[wi] + 1],
                    )
                    jb = wpool.tile([128, w], BF16, name="junkb")
                    nc.vector.scalar_tensor_tensor(
                        out=jb[:],
                        in0=bt[:, lo:lo + w],
                        scalar=0.0,
                        in1=m[:],
                        op0=AL.add,
                        op1=AL.mult,
                        accum_out=accD[:, colB[wi]:colB[wi] + 1],
                    )



def build_program(pad, relu_tiles=RELU_TILES, repeats=1):
    assert 0 < pad <= 96, f"label run too long for band kernel (pad={pad})"
    nc = bacc.Bacc()
    xt_d = nc.dram_tensor("xt", [2, 128, N_TOTAL], BF16, kind="ExternalInput")
    tcol_d = nc.dram_tensor("tcol", [N_TOTAL], F32, kind="ExternalInput")

    wins, a_len = _band_windows(pad)
    order = [(mt, dt) for mt in range(M_TILES) for dt in range(N_DT)]
    n_tiles = len(order)
    forced = {(mt, dt) for (mt, dt, _, _, _) in wins}
    nonforced = [td for td in order if td not in forced]
    rset = {
        nonforced[(i * len(nonforced)) // relu_tiles] for i in range(relu_tiles)
    } if relu_tiles else set()

    # accD columns: one per C-tile (b-sum), then 2 per band window.
    # accE columns: 2 per R-tile (relu-sum, sign-sum).
    cD = 0
    cE = 0
    stt_col = {}
    colR = {}
    colS = {}
    for td in order:
        if td in rset:
            colR[td] = cE
            colS[td] = cE + 1
            cE += 2
        else:
            stt_col[td] = cD
            cD += 1
    colA = {}
    colB = {}
    for wi in range(len(wins)):
        colA[wi] = cD
        colB[wi] = cD + 1
        cD += 2
    CD, CE = cD, cE

    out_d = nc.dram_tensor("out", [128, CD + CE], F32, kind="ExternalOutput")

    wins_by_td = {}
    for wi, (mt, dt, lo, w, tco) in enumerate(wins):
        wins_by_td.setdefault((mt, dt), []).append((wi, lo, w, tco))

    AL = mybir.AluOpType
    ACT = mybir.ActivationFunctionType

    with tile.TileContext(nc) as tc:
        with (
            tc.tile_pool(name="resident", bufs=1) as rpool,
            tc.tile_pool(name="psum", bufs=4, space="PSUM") as psum,
            tc.tile_pool(name="scopy", bufs=4) as spool,
            tc.tile_pool(name="bt", bufs=3) as bpool,
            tc.tile_pool(name="band", bufs=2) as wpool,
        ):
            # resident bf16 X^T (rolled), K split into 2 partition tiles
            xk = [rpool.tile([128, N_TOTAL], BF16, name=f"xk{k}") for k in range(2)]
            for ch in range(4):
                sl = slice(ch * 2048, (ch + 1) * 2048)
                for k in range(2):
                    nc.sync.dma_start(out=xk[k][:, sl], in_=xt_d[k, :, sl])

            # label tiles
            tcol_ap = tcol_d[:]
            tcb = rpool.tile([128, a_len + pad], F32, name="tcb")
            nc.sync.dma_start(
                out=tcb[:, 0:a_len],
                in_=bass.AP(tensor=tcol_ap.tensor, offset=0, ap=[[0, 128], [1, a_len]]),
            )
            nc.sync.dma_start(
                out=tcb[:, a_len:a_len + pad],
                in_=bass.AP(
                    tensor=tcol_ap.tensor,
                    offset=N_TOTAL - pad,
                    ap=[[0, 128], [1, pad]],
                ),
            )
            trows = rpool.tile([128, M_TILES], F32, name="trows")
            nc.sync.dma_start(
                out=trows[:],
                in_=bass.AP(
                    tensor=tcol_ap.tensor, offset=0, ap=[[1, 128], [128, M_TILES]]
                ),
            )

            accD = rpool.tile([128, CD], F32, name="accD")
            accE = rpool.tile([128, max(CE, 1)], F32, name="accE")
            nc.vector.memset(accD[:], 0.0)
            nc.vector.memset(accE[:], 0.0)
            bias_nm = rpool.tile([128, 1], F32, name="bias_nm")
            nc.vector.memset(bias_nm[:], -MARGIN)

            import contextlib
            loop_cm = tc.For_i(0, repeats, 1) if repeats > 1 else contextlib.nullcontext()
            with loop_cm:
                _main_body(nc, tc, psum, spool, bpool, wpool, xk, tcb, trows,
                           accD, accE, bias_nm, rset, stt_col, colR, colS,
                           colA, colB, wins_by_td, AL, ACT)

            nc.sync.dma_start(out=out_d[:, 0:CD], in_=accD[:])
            if CE:
                nc.sync.dma_start(out=out_d[:, CD:CD + CE], in_=accE[:])


    meta = dict(
        CD=CD, CE=CE, n_relu=len(rset),
        stt_cols=sorted(stt_col.values()),
        a_cols=sorted(colA.values()),
        b_cols=sorted(colB.values()),
        r_cols=sorted(colR.values()),
        s_cols=sorted(colS.values()),
    )
    return nc, meta


def host_reduce(out_arr, meta):
    """out_arr: [128, CD+CE] f32 from one core -> fp64 partial of loss*n."""
    a = out_arr.astype(np.float64)
    d = a[:, :meta["CD"]]
    tot = d[:, meta["stt_cols"]].sum()
    tot += d[:, meta["a_cols"]].sum()
    tot -= d[:, meta["b_cols"]].sum()
    if meta["CE"]:
        e = a[:, meta["CD"]:meta["CD"] + meta["CE"]]
        tot += e[:, meta["r_cols"]].sum()
        npix = meta["n_relu"] * 128 * DT_W
        tot += MARGIN * 0.5 * (npix + e[:, meta["s_cols"]].sum())
    return tot


def prepare_inputs(inputs, targets):
    X = np.asarray(inputs, dtype=np.float32)
    t = np.asarray(targets).astype(np.int64).reshape(-1)
    n, d = X.shape
    assert (n, d) == (N_TOTAL, D), f"kernel hardcoded for {N_TOTAL}x{D}, got {n}x{d}"
    perm = np.argsort(t, kind="stable")
    ts_ = t[perm]
    tf = ts_.astype(np.float32)
    bounds = np.flatnonzero(np.concatenate(([True], ts_[1:] != ts_[:-1], [True])))
    maxrun = int(np.diff(bounds).max())
    pad = int(-(-max(32, maxrun - 1) // 32) * 32)
    XT = np.ascontiguousarray(X[perm].T).astype(ml_dtypes.bfloat16)
    xt_full = XT.reshape(2, 128, N_TOTAL)
    in_maps = []
    for c in range(N_CORES):
        r = -c * ROWS
        in_maps.append({
            "xt": np.ascontiguousarray(np.roll(xt_full, r, axis=2)),
            "tcol": np.ascontiguousarray(np.roll(tf, r)),
        })
    return in_maps, pad


def run(inputs, targets, trace=False):
    in_maps, pad = prepare_inputs(inputs, targets)
    nc, meta = build_program(pad)
    nc.finalize()
    res = run_bass_kernel_spmd(
        nc, in_maps, core_ids=list(range(N_CORES)), trace=trace
    )
    total = 0.0
    for r in res.results:
        total += host_reduce(r["out"], meta)
    return np.asarray(total / N_TOTAL, dtype=np.float32), res


def kernel(inputs, targets):
    val, _ = run(inputs, targets, trace=False)
    return val



# revision 14
# speedup vs baseline: 6.5409x; 6.5409x over previous
"""Contrastive loss on 8 Trainium2 NeuronCores (Bass/Tile).

loss * n = sum_ij [ same_ij * (s<1)(1-s) + (1-same_ij) * (s>0.3) * s ],
s = <x_i, x_j>.

Exact decomposition (rows pre-sorted by label so same-label pairs live in
a band |i-j| < pad):
    loss * n = sum_ij b(s) + sum_ij same_ij * (relu(1-s) - b(s)),
    b(s) = (s > 0.3) * s.

Strategy (vs. the full-S baseline):
  * S is symmetric -> only the upper triangle of the 16x16 grid of
    512-wide blocks is computed: 136 blocks instead of 256.  Core c is
    assigned triangle rows c and 15-c: (16-c) + (c+1) = 17 blocks for
    every core, including exactly two diagonal blocks.  Off-diagonal
    blocks enter the total with weight 2, diagonal blocks with weight 1.
  * One SPMD program: the host gathers, per core, per task slot
    t in 0..16, the lhsT block [128, 2, 512] and rhs block [128, 2, 512]
    into two resident fp8 tensors, so the program is task-index uniform.
    Slots 0/16 hold the two diagonal blocks, slots 1/15 the two
    band-corner blocks (r, r+1); the host also precomputes the
    label-equality masks for those four slots as bf16 tensors.
  * Matmul in fp8e4m3 with MatmulPerfMode.DoubleRow (K=256 in one pass;
    host-checked loss error ~8e-4, well inside the 2e-2 gate).
  * The PSUM->accumulator drain (the real bottleneck) is split across
    all three elementwise engines: per [128, 1024] S unit either
      C: ScalarE copy to SBUF bf16 + DVE fused (S>m)*S row-accumulated
         (4x DVE mode on bf16 SBUF operands), or
      V: DVE scalar_tensor_tensor directly on the PSUM tile, or
      P: Pool (gpsimd) scalar_tensor_tensor directly on the PSUM tile.
    Band-correction slots are pinned to mode C (they need S in SBUF).
  * Host: fp64 sum of per-core accumulator columns with weights 1/2,
    divide by n.
"""

import numpy as np
import ml_dtypes

import concourse.bass as bass
import concourse.mybir as mybir
from concourse import bacc
import concourse.tile as tile
from concourse.bass_utils import run_bass_kernel_spmd

N_TOTAL = 8192
D = 256
N_CORES = 8
GB = 512                      # grid block width
G = N_TOTAL // GB             # 16 col/row blocks
NS = 17                       # task slots per core
ST = 4                        # 128-row stripes per block
MARGIN = 0.3
F32 = mybir.dt.float32
BF16 = mybir.dt.bfloat16
FP8 = mybir.dt.float8e4

# drain-unit pairing of the 17 slots: same-weight pairs -> one accum col
# per (unit, stripe). unit 0 = both diagonal blocks (weight 1), unit 1 =
# both band-corner blocks (weight 2); slot 14 is the odd narrow unit.
UNIT_PAIRS = [(0, 16), (1, 15), (2, 3), (4, 5), (6, 7), (8, 9), (10, 11),
              (12, 13)]
NARROW_SLOT = 14
UNIT_W = [1.0, 2.0] + [2.0] * 6          # weight per wide unit
# PSUM->SBUF copy engine per (unit, stripe): 'A' ScalarE activation,
# 'V' DVE tensor_copy. (The HW verifier forbids dual-PSUM-operand stt,
# and GpSimd cannot access PSUM at all, so every unit is copied to SBUF
# bf16 by Act or DVE and then reduced by one fused stt -- on DVE (4x
# mode, 'd') or on the otherwise-idle Pool engine ('p').)
DRAIN_MODES = [
    "AAAA",   # unit 0 (diag slots 0,16; band corrections read the copy)
    "AAAA",   # unit 1 (corner slots 1,15)
    "AAAA",
    "AAAA",
    "AAAA",
    "AAVV",
    "VVVV",
    "VVVV",
]
NARROW_MODES = "VVVV"         # slot 14's four [128,512] drains
# stt engine: 'd' DVE (4x bf16). Pool cannot run TensorScalarPtr on TRN2
# and shares its SBUF port with DVE anyway.
STT_MODES = ["dddd"] * 8
NARROW_STT = "dddd"


def _windows(pad):
    """Band windows for a diagonal block, per stripe: (lo, hi) col range
    within the 512-wide block plus running offset into the mask tensor."""
    wins = []
    off = 0
    for st in range(ST):
        lo = max(0, 128 * st - pad)
        hi = min(GB, 128 * st + 128 + pad)
        wins.append((st, lo, hi - lo, off))
        off += hi - lo
    return wins, off            # off = 512 + 6*pad


def build_program(pad, repeats=1):
    assert 0 < pad <= 96
    nc = bacc.Bacc()
    LW = NS * GB                # 8704 cols in lhs/rhs tensors
    lhs_d = nc.dram_tensor("lhs8", [128, 2, LW], FP8, kind="ExternalInput")
    rhs_d = nc.dram_tensor("rhs8", [128, 2, LW], FP8, kind="ExternalInput")
    wins, wlen = _windows(pad)
    MKW = 2 * wlen + 2 * pad    # masks: slot0 | slot1 | slot15 | slot16
    mk_d = nc.dram_tensor("mk", [128, MKW], BF16, kind="ExternalInput")
    m_off = {0: 0, 1: wlen, 15: wlen + pad, 16: wlen + 2 * pad}

    # accumulator columns: 8 wide units x 4 stripes, 4 narrow, 10 corr
    n_base = len(UNIT_PAIRS) * ST
    n_narrow = ST
    corr_cols = {}              # (slot, st) -> col
    cc = n_base + n_narrow
    for slot in (0, 16):
        for st in range(ST):
            corr_cols[(slot, st)] = cc
            cc += 1
    for slot in (1, 15):
        corr_cols[(slot, 3)] = cc
        cc += 1
    CD = cc
    out_d = nc.dram_tensor("out", [128, CD], F32, kind="ExternalOutput")

    AL = mybir.AluOpType
    ACT = mybir.ActivationFunctionType
    DR = mybir.MatmulPerfMode.DoubleRow

    with tile.TileContext(nc) as tc:
        with (
            tc.tile_pool(name="resident", bufs=1) as rpool,
            tc.tile_pool(name="psum", bufs=3, space="PSUM") as psum,
            tc.tile_pool(name="psumn", bufs=2, space="PSUM") as psumn,
            tc.tile_pool(name="scopy", bufs=4) as spool,
            tc.tile_pool(name="junk", bufs=4) as jpool,
            tc.tile_pool(name="band", bufs=2) as wpool,
        ):
            lhs8 = rpool.tile([128, 2, LW], FP8, name="lhs8")
            rhs8 = rpool.tile([128, 2, LW], FP8, name="rhs8")
            for chunk in range(4):
                sl = slice(chunk * (LW // 4), (chunk + 1) * (LW // 4))
                nc.sync.dma_start(out=lhs8[:, :, sl], in_=lhs_d[:, :, sl])
                nc.sync.dma_start(out=rhs8[:, :, sl], in_=rhs_d[:, :, sl])
            mk = rpool.tile([128, MKW], BF16, name="mk")
            nc.sync.dma_start(out=mk[:], in_=mk_d[:])

            accD = rpool.tile([128, CD], F32, name="accD")
            nc.vector.memset(accD[:], 0.0)

            def mm(dst, slot, st):
                nc.tensor.matmul(
                    dst,
                    lhs8[:, :, slot * GB + st * 128: slot * GB + (st + 1) * 128],
                    rhs8[:, :, slot * GB: (slot + 1) * GB],
                    start=True, stop=True, perf_mode=DR,
                )

            def drain(mode, stt, T, width, col):
                """Copy PSUM tile to SBUF bf16 on Act or DVE, then one
                fused (S>m)*S row-accumulated stt on DVE or Pool.
                Returns (Sb, bt) for the band corrections."""
                Sb = spool.tile([128, width], BF16, name="scp")
                if mode == "A":
                    nc.scalar.activation(
                        out=Sb[:], in_=T[:], func=ACT.Copy,
                        bias=0.0, scale=1.0,
                    )
                else:
                    nc.vector.tensor_copy(out=Sb[:], in_=T[:])
                bt = jpool.tile([128, width], BF16, name="bt")
                eng = nc.gpsimd if stt == "p" else nc.vector
                eng.scalar_tensor_tensor(
                    out=bt[:], in0=Sb[:], scalar=MARGIN,
                    in1=Sb[:], op0=AL.is_gt, op1=AL.mult,
                    accum_out=accD[:, col:col + 1],
                )
                return Sb, bt

            def corrections(slot, st, half, Sb, bt):
                """corr col += sum(mask * (relu(1-s) - b)) over the window."""
                if slot in (0, 16):
                    w_st, lo, w, moff = wins[st]
                    assert w_st == st
                elif st == 3:
                    lo, w, moff = 0, pad, 0
                else:
                    return
                sl = slice(half * GB + lo, half * GB + lo + w)
                at = wpool.tile([128, w], BF16, name="at")
                nc.scalar.activation(
                    out=at[:], in_=Sb[:, sl], func=ACT.Relu,
                    bias=1.0, scale=-1.0,
                )
                dt_ = wpool.tile([128, w], BF16, name="dt")
                nc.vector.tensor_sub(out=dt_[:], in0=at[:], in1=bt[:, sl])
                jc = wpool.tile([128, w], BF16, name="jc")
                msl = slice(m_off[slot] + moff, m_off[slot] + moff + w)
                col = corr_cols[(slot, st)]
                nc.vector.scalar_tensor_tensor(
                    out=jc[:], in0=dt_[:], scalar=0.0, in1=mk[:, msl],
                    op0=AL.add, op1=AL.mult,
                    accum_out=accD[:, col:col + 1],
                )

            def body():
                for st in range(ST):
                    for u, (p, q) in enumerate(UNIT_PAIRS):
                        T = psum.tile([128, 2 * GB], F32, name="S")
                        mm(T[:, 0:GB], p, st)
                        mm(T[:, GB:2 * GB], q, st)
                        col = u * ST + st
                        Sb, bt = drain(DRAIN_MODES[u][st], STT_MODES[u][st],
                                       T, 2 * GB, col)
                        if u in (0, 1):
                            corrections(p, st, 0, Sb, bt)
                            corrections(q, st, 1, Sb, bt)
                    # narrow slot
                    Tn = psumn.tile([128, GB], F32, name="Sn")
                    mm(Tn[:], NARROW_SLOT, st)
                    drain(NARROW_MODES[st], NARROW_STT[st], Tn, GB,
                          n_base + st)

            import contextlib
            loop_cm = tc.For_i(0, repeats, 1) if repeats > 1 else \
                contextlib.nullcontext()
            with loop_cm:
                body()

            nc.sync.dma_start(out=out_d[:], in_=accD[:])

    meta = dict(CD=CD, n_base=n_base, n_narrow=n_narrow,
                corr_cols=dict(corr_cols), pad=pad)
    return nc, meta


def task_slots(c):
    """Slot -> (row block, col block) for core c. Slots 0/16 diagonal,
    1/15 band-corner; the rest hold the remaining triangle blocks."""
    rA, rB = c, (G - 1) - c
    blocks = [(rA, j) for j in range(rA, G)] + \
             [(rB, j) for j in range(rB, G)]
    slots = {0: (rA, rA), 1: (rA, rA + 1), 16: (rB, rB)}
    if c >= 1:
        slots[15] = (rB, rB + 1)
    fixed = set(slots.values())
    rest = [blk for blk in blocks if blk not in fixed]
    free = [s for s in range(NS) if s not in slots]
    for s, blk in zip(free, rest, strict=True):
        slots[s] = blk
    return slots


def prepare_inputs(inputs, targets):
    X = np.asarray(inputs, dtype=np.float32)
    t = np.asarray(targets).astype(np.int64).reshape(-1)
    n, d = X.shape
    assert (n, d) == (N_TOTAL, D), f"kernel hardcoded for {N_TOTAL}x{D}"
    perm = np.argsort(t, kind="stable")
    ts_ = t[perm]
    bounds = np.flatnonzero(
        np.concatenate(([True], ts_[1:] != ts_[:-1], [True])))
    maxrun = int(np.diff(bounds).max())
    pad = int(-(-max(32, maxrun) // 32) * 32)
    XT = np.ascontiguousarray(X[perm].T).astype(ml_dtypes.float8_e4m3)
    # [128, 2, N]: partition lane p holds dims p (k0) and 128+p (k1)
    XK = XT.reshape(2, 128, N_TOTAL).transpose(1, 0, 2)
    tf = ts_.astype(np.float64)
    wins, wlen = _windows(pad)
    MKW = 2 * wlen + 2 * pad

    in_maps = []
    for c in range(N_CORES):
        slots = task_slots(c)
        lhs = np.zeros((128, 2, NS * GB), dtype=XK.dtype)
        rhs = np.zeros((128, 2, NS * GB), dtype=XK.dtype)
        for s in range(NS):
            r, j = slots[s]
            lhs[:, :, s * GB:(s + 1) * GB] = XK[:, :, r * GB:(r + 1) * GB]
            rhs[:, :, s * GB:(s + 1) * GB] = XK[:, :, j * GB:(j + 1) * GB]
        mkv = np.zeros((128, MKW), dtype=np.float64)
        off = 0
        for slot in (0, 1, 15, 16):
            if slot in (0, 16):
                r, j = slots[slot]
                for st, lo, w, moff in wins:
                    rows = tf[r * GB + st * 128: r * GB + (st + 1) * 128]
                    cols = tf[j * GB + lo: j * GB + lo + w]
                    mkv[:, off + moff: off + moff + w] = (
                        rows[:, None] == cols[None, :])
                off += wlen
            else:
                if slot in slots:
                    r, j = slots[slot]
                    rows = tf[r * GB + 3 * 128: r * GB + 4 * 128]
                    cols = tf[j * GB: j * GB + pad]
                    mkv[:, off: off + pad] = (rows[:, None] == cols[None, :])
                off += pad
        in_maps.append({
            "lhs8": lhs,
            "rhs8": rhs,
            "mk": mkv.astype(ml_dtypes.bfloat16),
        })
    return in_maps, pad


def host_reduce(out_arr, meta):
    """out_arr: [128, CD] f32 from one core -> fp64 partial of loss*n."""
    a = out_arr.astype(np.float64)
    tot = 0.0
    for u, w in enumerate(UNIT_W):
        tot += w * a[:, u * ST:(u + 1) * ST].sum()
    tot += 2.0 * a[:, meta["n_base"]:meta["n_base"] + ST].sum()
    for (slot, st), col in meta["corr_cols"].items():
        w = 1.0 if slot in (0, 16) else 2.0
        tot += w * a[:, col].sum()
    return tot


def run(inputs, targets, trace=False):
    in_maps, pad = prepare_inputs(inputs, targets)
    nc, meta = build_program(pad)
    nc.finalize()
    res = run_bass_kernel_spmd(
        nc, in_maps, core_ids=list(range(N_CORES)), trace=trace
    )
    total = 0.0
    for r in res.results:
        total += host_reduce(r["out"], meta)
    return np.asarray(total / N_TOTAL, dtype=np.float32), res


def kernel(inputs, targets):
    val, _ = run(inputs, targets, trace=False)
    return val


# revision 21
# speedup vs baseline: 6.5967x; 1.0085x over previous
"""Contrastive loss on 8 Trainium2 NeuronCores (Bass/Tile).

loss * n = sum_ij [ same_ij * (s<1)(1-s) + (1-same_ij) * (s>0.3) * s ],
s = <x_i, x_j>.

Exact decomposition (rows pre-sorted by label so same-label pairs live in
a band |i-j| < pad):
    loss * n = sum_ij b(s) + sum_ij same_ij * (relu(1-s) - b(s)),
    b(s) = (s > 0.3) * s.

Strategy (vs. the full-S baseline):
  * S is symmetric -> only the upper triangle of the 16x16 grid of
    512-wide blocks is computed: 136 blocks instead of 256.  Core c is
    assigned triangle rows c and 15-c: (16-c) + (c+1) = 17 blocks for
    every core, including exactly two diagonal blocks.  Off-diagonal
    blocks enter the total with weight 2, diagonal blocks with weight 1.
  * One SPMD program: the host gathers, per core, per task slot
    t in 0..16, the lhsT block [128, 2, 512] and rhs block [128, 2, 512]
    into two resident fp8 tensors, so the program is task-index uniform.
    Slots 0/16 hold the two diagonal blocks, slots 1/15 the two
    band-corner blocks (r, r+1); the host also precomputes the
    label-equality masks for those four slots as bf16 tensors.
  * Matmul in fp8e4m3 with MatmulPerfMode.DoubleRow (K=256 in one pass;
    host-checked loss error ~8e-4, well inside the 2e-2 gate).
  * The PSUM->accumulator drain (the real bottleneck) is split across
    all three elementwise engines: per [128, 1024] S unit either
      C: ScalarE copy to SBUF bf16 + DVE fused (S>m)*S row-accumulated
         (4x DVE mode on bf16 SBUF operands), or
      V: DVE scalar_tensor_tensor directly on the PSUM tile, or
      P: Pool (gpsimd) scalar_tensor_tensor directly on the PSUM tile.
    Band-correction slots are pinned to mode C (they need S in SBUF).
  * Host: fp64 sum of per-core accumulator columns with weights 1/2,
    divide by n.
"""

import numpy as np
import ml_dtypes

import concourse.bass as bass
import concourse.mybir as mybir
from concourse import bacc
import concourse.tile as tile
from concourse.bass_utils import run_bass_kernel_spmd

N_TOTAL = 8192
D = 256
N_CORES = 8
GB = 512                      # grid block width
G = N_TOTAL // GB             # 16 col/row blocks
NS = 17                       # task slots per core
ST = 4                        # 128-row stripes per block
MARGIN = 0.3
F32 = mybir.dt.float32
BF16 = mybir.dt.bfloat16
FP8 = mybir.dt.float8e4

# drain-unit pairing of the 17 slots: same-weight pairs -> one accum col
# per (unit, stripe). unit 0 = both diagonal blocks (weight 1), unit 1 =
# both band-corner blocks (weight 2); slot 14 is the odd narrow unit.
UNIT_PAIRS = [(0, 16), (1, 15), (2, 3), (4, 5), (6, 7), (8, 9), (10, 11),
              (12, 13)]
NARROW_SLOT = 14
UNIT_W = [1.0, 2.0] + [2.0] * 6          # weight per wide unit
# PSUM->SBUF copy engine per (unit, stripe): 'A' ScalarE activation,
# 'V' DVE tensor_copy. (The HW verifier forbids dual-PSUM-operand stt,
# and GpSimd cannot access PSUM at all, so every unit is copied to SBUF
# bf16 by Act or DVE and then reduced by one fused stt -- on DVE (4x
# mode, 'd') or on the otherwise-idle Pool engine ('p').)
# 'A' = Act relu-copy (accum sum(relu(s-m))) + DVE 4x count;
# 'V' = DVE max-copy tensor_scalar (accum sum(max(s,m))) + DVE 4x count;
# band unit-stripes (unit 0 all stripes, unit 1 stripe 3) are forced to
# plain Act copy + window zeroing + max/count tensor_scalar pair.
DRAIN_MODES = [
    "AAAA",   # unit 0 (diag slots 0,16): band
    "AAAA",   # unit 1 (corner slots 1,15): stripe 3 band
    "AAAA",
    "AAAA",
    "AAAA",
    "AAAA",
    "AAVV",
    "VVVV",
]
NARROW_MODES = "AVVV"         # slot 14's four [128,512] drains


def unit_kind(u, st):
    """'band' (plain copy + masked-window zeroing, max/count pair),
    'relu' (Act relu-copy) or 'max' (DVE max-copy)."""
    if u == 0 or (u == 1 and st == 3):
        return "band"
    return "relu" if DRAIN_MODES[u][st] == "A" else "max"


def _windows(pad):
    """Band windows for a diagonal block, per stripe: (lo, hi) col range
    within the 512-wide block plus running offset into the mask tensor."""
    wins = []
    off = 0
    for st in range(ST):
        lo = max(0, 128 * st - pad)
        hi = min(GB, 128 * st + 128 + pad)
        wins.append((st, lo, hi - lo, off))
        off += hi - lo
    return wins, off            # off = 512 + 6*pad


def build_program(pad, repeats=1, ablate=frozenset()):
    """ablate (timing experiments only, breaks math): 'nocorr' drop band
    corrections, 'nostt' drop the fused reductions, 'nocopy' drop the
    PSUM->SBUF copies (implies nostt/nocorr), 'nomm' drop matmuls,
    'privacc' one accumulator tile per op instead of accD columns."""
    assert 0 < pad <= 96
    nc = bacc.Bacc()
    LW = NS * GB                # 8704 cols in lhs/rhs tensors
    lhs_d = nc.dram_tensor("lhs8", [128, 2, LW], FP8, kind="ExternalInput")
    rhs_d = nc.dram_tensor("rhs8", [128, 2, LW], FP8, kind="ExternalInput")
    wins, wlen = _windows(pad)
    MKW = 2 * wlen + 2 * pad    # masks: slot0 | slot1 | slot15 | slot16
    mk_d = nc.dram_tensor("mk", [128, MKW], BF16, kind="ExternalInput")
    m_off = {0: 0, 1: wlen, 15: wlen + pad, 16: wlen + 2 * pad}

    # accumulator columns: 2 per unit-stripe (sum + count), then narrow,
    # then one per correction window
    n_base = 2 * len(UNIT_PAIRS) * ST          # 64
    n_narrow = 2 * ST                          # 8
    corr_cols = {}              # (slot, st) -> col
    cc = n_base + n_narrow
    for slot in (0, 16):
        for st in range(ST):
            corr_cols[(slot, st)] = cc
            cc += 1
    for slot in (1, 15):
        corr_cols[(slot, 3)] = cc
        cc += 1
    CD = cc
    out_d = nc.dram_tensor("out", [128, CD], F32, kind="ExternalOutput")

    AL = mybir.AluOpType
    ACT = mybir.ActivationFunctionType
    DR = mybir.MatmulPerfMode.DoubleRow

    with tile.TileContext(nc) as tc:
        with (
            tc.tile_pool(name="resident", bufs=1) as rpool,
            tc.tile_pool(name="psum", bufs=3, space="PSUM") as psum,
            tc.tile_pool(name="psumn", bufs=2, space="PSUM") as psumn,
            tc.tile_pool(name="scopy", bufs=4) as spool,
            tc.tile_pool(name="junk", bufs=4) as jpool,
            tc.tile_pool(name="band", bufs=2) as wpool,
        ):
            lhs8 = rpool.tile([128, 2, LW], FP8, name="lhs8")
            rhs8 = rpool.tile([128, 2, LW], FP8, name="rhs8")
            for chunk in range(4):
                sl = slice(chunk * (LW // 4), (chunk + 1) * (LW // 4))
                nc.sync.dma_start(out=lhs8[:, :, sl], in_=lhs_d[:, :, sl])
                nc.sync.dma_start(out=rhs8[:, :, sl], in_=rhs_d[:, :, sl])
            mk = rpool.tile([128, MKW], BF16, name="mk")
            nc.sync.dma_start(out=mk[:], in_=mk_d[:])
            imk = rpool.tile([128, MKW], BF16, name="imk")
            nc.vector.tensor_scalar(
                out=imk[:], in0=mk[:], scalar1=0.5, scalar2=None,
                op0=mybir.AluOpType.is_lt,
            )
            bias_nm = rpool.tile([128, 1], F32, name="bias_nm")
            nc.vector.memset(bias_nm[:], -MARGIN)

            accD = rpool.tile([128, CD], F32, name="accD")
            nc.vector.memset(accD[:], 0.0)
            if "privacc" in ablate:
                acc_t = [rpool.tile([128, 1], F32, name=f"acc{i}")
                         for i in range(CD)]
                for a_ in acc_t:
                    nc.vector.memset(a_[:], 0.0)
                acc_ap = lambda col: acc_t[col][:]
            else:
                acc_ap = lambda col: accD[:, col:col + 1]

            def mm(dst, slot, st):
                nc.tensor.matmul(
                    dst,
                    lhs8[:, :, slot * GB + st * 128: slot * GB + (st + 1) * 128],
                    rhs8[:, :, slot * GB: (slot + 1) * GB],
                    start=True, stop=True, perf_mode=DR,
                )

            def drain(u_kind, T, width, col):
                """Drain one PSUM tile into accD[col] (sum) and
                accD[col+1] (count) per the unit kind. Returns the SBUF
                bf16 copy (plain S for 'band', relu(S-m) for 'relu',
                max(S,m) for 'max')."""
                Sb = spool.tile([128, width], BF16, name="scp")
                if u_kind == "band":
                    nc.scalar.activation(
                        out=Sb[:], in_=T[:], func=ACT.Copy,
                        bias=0.0, scale=1.0,
                    )
                    return Sb
                if u_kind == "relu":
                    nc.scalar.activation(
                        out=Sb[:], in_=T[:], func=ACT.Relu,
                        bias=bias_nm[:], scale=1.0,
                        accum_out=acc_ap(col),
                    )
                    cnt_thresh = 0.0
                else:
                    nc.vector.tensor_scalar(
                        out=Sb[:], in0=T[:], scalar1=MARGIN, scalar2=None,
                        op0=AL.max, op1=AL.add,
                        accum_out=acc_ap(col),
                    )
                    # max(s,m) stores bf16(0.3)=0.30078; threshold
                    # between that and the next bf16 value excludes
                    # clamped cells from the count
                    cnt_thresh = 0.3015
                jc = jpool.tile([128, width], BF16, name="cnt")
                nc.vector.tensor_scalar(
                    out=jc[:], in0=Sb[:], scalar1=cnt_thresh, scalar2=None,
                    op0=AL.is_gt, op1=AL.add,
                    accum_out=acc_ap(col + 1),
                )
                return Sb

            def band_finish(Sb, width, col, winlist):
                """winlist: (slot, st, half, lo, w, moff) windows in this
                unit. Zero same-label cells in Sb, accumulate
                sum(mask*relu(1-s)) per window, then the max/count pair
                over the whole (modified) tile."""
                for slot, st, half, lo, w, moff in winlist:
                    sl = slice(half * GB + lo, half * GB + lo + w)
                    msl = slice(m_off[slot] + moff, m_off[slot] + moff + w)
                    at = wpool.tile([128, w], BF16, name="at")
                    nc.scalar.activation(
                        out=at[:], in_=Sb[:, sl], func=ACT.Relu,
                        bias=1.0, scale=-1.0,
                    )
                    jw = wpool.tile([128, w], BF16, name="jw")
                    nc.vector.scalar_tensor_tensor(
                        out=jw[:], in0=at[:], scalar=0.0, in1=mk[:, msl],
                        op0=AL.add, op1=AL.mult,
                        accum_out=acc_ap(corr_cols[(slot, st)]),
                    )
                    nc.vector.tensor_tensor(
                        out=Sb[:, sl], in0=Sb[:, sl], in1=imk[:, msl],
                        op=AL.mult,
                    )
                jm = jpool.tile([128, width], BF16, name="jm")
                nc.vector.tensor_scalar(
                    out=jm[:], in0=Sb[:], scalar1=MARGIN, scalar2=None,
                    op0=AL.max, op1=AL.add,
                    accum_out=acc_ap(col),
                )
                jc = jpool.tile([128, width], BF16, name="cnt")
                nc.vector.tensor_scalar(
                    out=jc[:], in0=jm[:], scalar1=0.3015, scalar2=None,
                    op0=AL.is_gt, op1=AL.add,
                    accum_out=acc_ap(col + 1),
                )

            def body():
                for st in range(ST):
                    for u, (p, q) in enumerate(UNIT_PAIRS):
                        T = psum.tile([128, 2 * GB], F32, name="S")
                        if "nomm" not in ablate:
                            mm(T[:, 0:GB], p, st)
                            mm(T[:, GB:2 * GB], q, st)
                        if "nocopy" in ablate:
                            continue
                        col = 2 * (u * ST + st)
                        kind = unit_kind(u, st)
                        if kind == "band" and "nocorr" not in ablate:
                            Sb = drain("band", T, 2 * GB, col)
                            winlist = []
                            for slot, half in ((p, 0), (q, 1)):
                                if slot in (0, 16):
                                    w_st, lo, w, moff = wins[st]
                                    winlist.append((slot, st, half, lo, w,
                                                    moff))
                                else:
                                    winlist.append((slot, st, half, 0, pad,
                                                    0))
                            band_finish(Sb, 2 * GB, col, winlist)
                        else:
                            kk = kind if kind != "band" else "relu"
                            drain(kk, T, 2 * GB, col)
                    # narrow slot
                    Tn = psumn.tile([128, GB], F32, name="Sn")
                    if "nomm" not in ablate:
                        mm(Tn[:], NARROW_SLOT, st)
                    if "nocopy" not in ablate:
                        nk = "relu" if NARROW_MODES[st] == "A" else "max"
                        drain(nk, Tn, GB, n_base + 2 * st)

            import contextlib
            loop_cm = tc.For_i(0, repeats, 1) if repeats > 1 else \
                contextlib.nullcontext()
            with loop_cm:
                body()

            if "privacc" in ablate:
                for i in range(CD):
                    nc.sync.dma_start(out=out_d[:, i:i + 1], in_=acc_t[i][:])
            else:
                nc.sync.dma_start(out=out_d[:], in_=accD[:])

    meta = dict(CD=CD, n_base=n_base, n_narrow=n_narrow,
                corr_cols=dict(corr_cols), pad=pad)
    return nc, meta


def task_slots(c):
    """Slot -> (row block, col block) for core c. Slots 0/16 diagonal,
    1/15 band-corner; the rest hold the remaining triangle blocks."""
    rA, rB = c, (G - 1) - c
    blocks = [(rA, j) for j in range(rA, G)] + \
             [(rB, j) for j in range(rB, G)]
    slots = {0: (rA, rA), 1: (rA, rA + 1), 16: (rB, rB)}
    if c >= 1:
        slots[15] = (rB, rB + 1)
    fixed = set(slots.values())
    rest = [blk for blk in blocks if blk not in fixed]
    free = [s for s in range(NS) if s not in slots]
    for s, blk in zip(free, rest, strict=True):
        slots[s] = blk
    return slots


def prepare_inputs(inputs, targets):
    X = np.asarray(inputs, dtype=np.float32)
    t = np.asarray(targets).astype(np.int64).reshape(-1)
    n, d = X.shape
    assert (n, d) == (N_TOTAL, D), f"kernel hardcoded for {N_TOTAL}x{D}"
    perm = np.argsort(t, kind="stable")
    ts_ = t[perm]
    bounds = np.flatnonzero(
        np.concatenate(([True], ts_[1:] != ts_[:-1], [True])))
    maxrun = int(np.diff(bounds).max())
    pad = int(-(-max(32, maxrun) // 32) * 32)
    XT = np.ascontiguousarray(X[perm].T).astype(ml_dtypes.float8_e4m3)
    # [128, 2, N]: partition lane p holds dims p (k0) and 128+p (k1)
    XK = XT.reshape(2, 128, N_TOTAL).transpose(1, 0, 2)
    tf = ts_.astype(np.float64)
    wins, wlen = _windows(pad)
    MKW = 2 * wlen + 2 * pad

    in_maps = []
    for c in range(N_CORES):
        slots = task_slots(c)
        lhs = np.zeros((128, 2, NS * GB), dtype=XK.dtype)
        rhs = np.zeros((128, 2, NS * GB), dtype=XK.dtype)
        for s in range(NS):
            r, j = slots[s]
            lhs[:, :, s * GB:(s + 1) * GB] = XK[:, :, r * GB:(r + 1) * GB]
            rhs[:, :, s * GB:(s + 1) * GB] = XK[:, :, j * GB:(j + 1) * GB]
        mkv = np.zeros((128, MKW), dtype=np.float64)
        off = 0
        for slot in (0, 1, 15, 16):
            if slot in (0, 16):
                r, j = slots[slot]
                for st, lo, w, moff in wins:
                    rows = tf[r * GB + st * 128: r * GB + (st + 1) * 128]
                    cols = tf[j * GB + lo: j * GB + lo + w]
                    mkv[:, off + moff: off + moff + w] = (
                        rows[:, None] == cols[None, :])
                off += wlen
            else:
                if slot in slots:
                    r, j = slots[slot]
                    rows = tf[r * GB + 3 * 128: r * GB + 4 * 128]
                    cols = tf[j * GB: j * GB + pad]
                    mkv[:, off: off + pad] = (rows[:, None] == cols[None, :])
                off += pad
        in_maps.append({
            "lhs8": lhs,
            "rhs8": rhs,
            "mk": mkv.astype(ml_dtypes.bfloat16),
        })
    return in_maps, pad


def host_reduce(out_arr, meta):
    """out_arr: [128, CD] f32 from one core -> fp64 partial of loss*n."""
    a = out_arr.astype(np.float64)
    tot = 0.0
    for u, w in enumerate(UNIT_W):
        for st in range(ST):
            c0 = 2 * (u * ST + st)
            s0, s1 = a[:, c0].sum(), a[:, c0 + 1].sum()
            if unit_kind(u, st) == "relu":
                part = s0 + MARGIN * s1
            else:
                part = s0 + MARGIN * (s1 - 128.0 * 2 * GB)
            tot += w * part
    for st in range(ST):
        c0 = meta["n_base"] + 2 * st
        s0, s1 = a[:, c0].sum(), a[:, c0 + 1].sum()
        if NARROW_MODES[st] == "A":
            part = s0 + MARGIN * s1
        else:
            part = s0 + MARGIN * (s1 - 128.0 * GB)
        tot += 2.0 * part
    for (slot, st), col in meta["corr_cols"].items():
        w = 1.0 if slot in (0, 16) else 2.0
        tot += w * a[:, col].sum()
    return tot


def run(inputs, targets, trace=False):
    in_maps, pad = prepare_inputs(inputs, targets)
    nc, meta = build_program(pad)
    nc.finalize()
    res = run_bass_kernel_spmd(
        nc, in_maps, core_ids=list(range(N_CORES)), trace=trace
    )
    total = 0.0
    for r in res.results:
        total += host_reduce(r["out"], meta)
    return np.asarray(total / N_TOTAL, dtype=np.float32), res


def kernel(inputs, targets):
    val, _ = run(inputs, targets, trace=False)
    return val
